# revision 24
# baseline (speedup 1.0000x reference)
"""Trainium2 Bass kernel for DecodeBoxLayer (box -> 4 corner points).

Reference semantics, per box (y, x, h, w) int32:
    x1 = 2x ; x2 = 2(x+w) ; y1 = 2y ; y2 = 2(y+h)
    corners = [[x1,y1],[x2,y1],[x2,y2],[x1,y2]]   # [4, 2] int32

Full input : boxes   [64, 100000, 4] int32
Full output: corners [64, 100000, 4, 2] int32

Sharding: batch axis across 8 cores (8 batches/core = 800k boxes/core).

All inputs are < 1000 and all outputs are even integers < 4096, so 2-byte
dtypes are lossless on-device: the host converts input to fp16 (exact) and
upcasts the int16 output while sharding/unsharding, and the device moves
half the HBM bytes of the int32 formulation (per core: 6.4 MB read +
12.8 MB write).  Input/temps are fp16 so the Pool engine can run the add
(integer TensorTensor is unsupported on Pool); outputs are int16 because
DVE tensor_scalar is ~1.6x faster with an int16 destination than fp16.

Host input layout (per core, [128, 25000] int16): partition-major, then
per-tile blocks of [yx pairs | hw pairs] so device reads are dense.

Per-box output out[0..7] = [a,b,c,b,c,d,a,d] with a=2x, b=2y, c=2(x+w),
d=2(y+h), i.e. output pairs (a,b) (c,b) (c,d) (a,d) = 2*(x,y) 2*(u,y)
2*(u,v) 2*(x,v) with u=x+w, v=y+h.  A 4-wide temp t=[y,u,v,x] per box
plus the input itself covers every pair as an adjacent (fwd/rev) slice,
with nested-stride APs fusing the two cross pairs into one op:
    Pool: t{1,2}   = rev(yx) + rev(hw)      ([u,v])
    ACT : t{0},{3} = copy(yx)               (y, x)
    DVE : out{0,1} = 2*rev(yx)              ([a,b], reads input directly)
    DVE : out{4,5} = 2*t{1,2}               ([c,d])
    DVE : out{2,3},{6,7} = 2*t{1,0},{3,2}   (fused [c,b],[a,d])
Engine arithmetic is fp32 internally; all values < 2^12 so exact.

DMA: input loads on the SP hardware DGE ring, output stores on the ACT
hardware DGE ring.  Tile sizes [625, 4x1250, 625] shorten the pipeline
lead-in (first compute starts sooner) and tail (last store is small).
"""

import numpy as np

import concourse.bacc as bacc
import concourse.bass as bass
import concourse.mybir as mybir
from concourse.ap import AP
from concourse import tile
from concourse.bass_utils import run_bass_kernel_spmd

N_CORES = 8
BATCH, NBOX = 64, 100000
BOXES_PER_CORE = (BATCH // N_CORES) * NBOX  # 800000
P = 128
BOXES_PER_PART = BOXES_PER_CORE // P  # 6250
TILE_W = [625, 1250, 1250, 1250, 1250, 625]  # boxes per (partition, tile)
assert sum(TILE_W) == BOXES_PER_PART
IN_COLS = BOXES_PER_PART * 4  # 25000
OUT_COLS = BOXES_PER_PART * 8  # 50000

IN_NAME = "boxes_in"
OUT_NAME = "corners_out"


def _sub_ap(full, extra_off, dims):
    """Replace the free dims of a full-tile AP with explicit [stride, count]s."""
    part = [list(full.ap[0])]
    return AP(full.tensor, full.offset + extra_off, part + [list(d) for d in dims])


def build_bass():
    nc = bacc.Bacc(None, target_bir_lowering=False, num_devices=N_CORES)
    inp = nc.declare_dram_parameter(IN_NAME, [P, IN_COLS], mybir.dt.int16, isOutput=False)
    outp = nc.declare_dram_parameter(OUT_NAME, [P, OUT_COLS], mybir.dt.int16, isOutput=True)

    with tile.TileContext(nc) as tc:
        with (
            tc.tile_pool(name="io_in", bufs=5) as pin,
            tc.tile_pool(name="io_out", bufs=5) as pout,
            tc.tile_pool(name="tmp", bufs=3) as ptmp,
        ):
            off = 0
            for ti, W in enumerate(TILE_W):
                tin = pin.tile([P, W * 4], mybir.dt.int16)
                load_eng = nc.sync if ti % 2 == 0 else nc.scalar
                load_eng.dma_start(tin[:], inp[:, off * 4 : off * 4 + W * 4])
                yx = tin[:, 0 : W * 2]
                hw = tin[:, W * 2 : W * 4]
                yx_rev = _sub_ap(yx, 1, [[2, W], [-1, 2]])  # [x, y] per box
                hw_rev = _sub_ap(hw, 1, [[2, W], [-1, 2]])  # [w, h] per box

                t4 = ptmp.tile([P, W * 4], mybir.dt.int16)
                # t per box = [y, u, v, x]
                nc.vector.tensor_add(
                    _sub_ap(t4[:], 1, [[4, W], [1, 2]]), yx_rev, hw_rev
                )  # t{1,2} = [x+w, y+h] = [u, v]
                nc.scalar.copy(
                    _sub_ap(t4[:], 0, [[4, W], [3, 2]]),
                    yx.rearrange("p (w c) -> p w c", c=2),
                )  # t{0},{3} = [y, x]

                tout = pout.tile([P, W * 8], mybir.dt.int16)
                # out{0,1} = 2*[x, y] = [a, b]   (ACT, reads the input tile)
                nc.scalar.mul(
                    _sub_ap(tout[:], 0, [[8, W], [1, 2]]), yx_rev, 2.0
                )
                # out{4,5} = 2*[u, v] = [c, d]
                nc.vector.tensor_scalar_mul(
                    _sub_ap(tout[:], 4, [[8, W], [1, 2]]),
                    _sub_ap(t4[:], 1, [[4, W], [1, 2]]),
                    2,
                )
                # out{2,3},{6,7} = 2*[u,y],[x,v] = [c,b],[a,d]  (fused rev pairs)
                nc.vector.tensor_scalar_mul(
                    _sub_ap(tout[:], 2, [[8, W], [4, 2], [1, 2]]),
                    _sub_ap(t4[:], 1, [[4, W], [2, 2], [-1, 2]]),
                    2,
                )

                # alternate store rings (SP / ACT HW DGE) to overlap ring gaps
                store_eng = nc.scalar if ti % 2 == 0 else nc.sync
                store_eng.dma_start(outp[:, off * 8 : off * 8 + W * 8], tout[:])
                off += W
    nc.compile()
    _strip_entry_barrier(nc)
    return nc


def _strip_entry_barrier(nc):
    """Drop the framework's const-AP all-engine barrier from the entry block.

    Bass.__init__ emits const-AP memsets followed by an all-engine barrier
    (drain + event-sem per engine on the barrier_* gather/release sems).
    This kernel never reads the const APs and all of its own ordering is
    semaphore-based from zero-initialized sems, so the entry rendezvous only
    delays the first load DMA (~2us, gated by the PE warm-up). Only the
    entry block is touched; the tail barriers keep their instructions.
    """
    blk = nc.m.functions[0].blocks[0]
    il = blk.instructions
    keep = []
    dropped = 0
    for ins in il:
        si = getattr(ins, "sync_info", None)
        names = []
        if si is not None:
            names = [w.ant_name or "" for w in si.on_wait] + [
                u.ant_name or "" for u in si.on_update
            ]
        if any(n.startswith("barrier_Pool_Activation_PE_DVE_SP") for n in names):
            dropped += 1
            continue
        keep.append(ins)
    assert dropped == 10, f"expected 10 entry-barrier insts, found {dropped}"
    blk.instructions = keep


_NC_CACHE = []


def _get_nc():
    if not _NC_CACHE:
        _NC_CACHE.append(build_bass())
    return _NC_CACHE[0]


def shard_inputs(boxes: np.ndarray) -> list[dict[str, np.ndarray]]:
    boxes = np.asarray(boxes).astype(np.int16)  # lossless: values in [0, 1000)
    # Per-tile blocked layout: [yx pairs | hw pairs] so device reads are dense.
    v = boxes.reshape(N_CORES, P, BOXES_PER_PART, 2, 2)
    parts = []
    off = 0
    for W in TILE_W:
        blk = v[:, :, off : off + W]  # [8, 128, W, 2, 2]
        parts.append(blk[..., 0, :].reshape(N_CORES, P, W * 2))
        parts.append(blk[..., 1, :].reshape(N_CORES, P, W * 2))
        off += W
    shards = np.ascontiguousarray(np.concatenate(parts, axis=-1))
    return [{IN_NAME: shards[c]} for c in range(N_CORES)]


def unshard_output(per_core: list[np.ndarray]) -> np.ndarray:
    out = np.stack([np.asarray(r) for r in per_core])  # [8, 128, 50000] int16
    return out.reshape(BATCH, NBOX, 4, 2).astype(np.int32)


def kernel(boxes: np.ndarray, **_run_kwargs) -> np.ndarray:
    nc = _get_nc()
    in_maps = shard_inputs(boxes)
    res = run_bass_kernel_spmd(nc, in_maps, list(range(N_CORES)), **_run_kwargs)
    out = unshard_output([res.results[c][OUT_NAME] for c in range(N_CORES)])
    if _run_kwargs:
        kernel.last_results = res
    return out


# revision 27
# speedup vs baseline: 1.1628x; 1.1628x over previous
"""Trainium2 Bass kernel for DecodeBoxLayer (box -> 4 corner points).

Reference semantics, per box (y, x, h, w) int32:
    x1 = 2x ; x2 = 2(x+w) ; y1 = 2y ; y2 = 2(y+h)
    corners = [[x1,y1],[x2,y1],[x2,y2],[x1,y2]]   # [4, 2] int32

Full input : boxes   [64, 100000, 4] int32
Full output: corners [64, 100000, 4, 2] int32

Sharding: batch axis across 8 cores (8 batches/core = 800k boxes/core).

All inputs are < 1000 and all outputs are even integers < 4096, so 2-byte
dtypes are lossless on-device: the host converts input to fp16 (exact) and
upcasts the int16 output while sharding/unsharding, and the device moves
half the HBM bytes of the int32 formulation (per core: 6.4 MB read +
12.8 MB write).  Input/temps are fp16 so the Pool engine can run the add
(integer TensorTensor is unsupported on Pool); outputs are int16 because
DVE tensor_scalar is ~1.6x faster with an int16 destination than fp16.

Host input layout (per core, [128, 25000] int16): partition-major, then
per-tile blocks of [yx pairs | hw pairs] so device reads are dense.

Per-box output out[0..7] = [a,b,c,b,c,d,a,d] with a=2x, b=2y, c=2(x+w),
d=2(y+h), i.e. output pairs (a,b) (c,b) (c,d) (a,d) = 2*(x,y) 2*(u,y)
2*(u,v) 2*(x,v) with u=x+w, v=y+h.  A 4-wide temp t=[y,u,v,x] per box
plus the input itself covers every pair as an adjacent (fwd/rev) slice,
with nested-stride APs fusing the two cross pairs into one op:
    Pool: t{1,2}   = rev(yx) + rev(hw)      ([u,v])
    ACT : t{0},{3} = copy(yx)               (y, x)
    DVE : out{0,1} = 2*rev(yx)              ([a,b], reads input directly)
    DVE : out{4,5} = 2*t{1,2}               ([c,d])
    DVE : out{2,3},{6,7} = 2*t{1,0},{3,2}   (fused [c,b],[a,d])
Engine arithmetic is fp32 internally; all values < 2^12 so exact.

DMA: input loads on the SP hardware DGE ring, output stores on the ACT
hardware DGE ring.  Tile sizes [625, 4x1250, 625] shorten the pipeline
lead-in (first compute starts sooner) and tail (last store is small).
"""

import numpy as np

import concourse.bacc as bacc
import concourse.bass as bass
import concourse.mybir as mybir
from concourse.ap import AP
from concourse import tile
from concourse.bass_utils import run_bass_kernel_spmd

N_CORES = 8
BATCH, NBOX = 64, 100000
BOXES_PER_CORE = (BATCH // N_CORES) * NBOX  # 800000
P = 128
BOXES_PER_PART = BOXES_PER_CORE // P  # 6250
TILE_W = [625, 1250, 1250, 1250, 1250, 625]  # boxes per (partition, tile)
assert sum(TILE_W) == BOXES_PER_PART
IN_COLS = BOXES_PER_PART * 4  # 25000
OUT_COLS = BOXES_PER_PART * 8  # 50000

IN_NAME = "boxes_in"
OUT_NAME = "corners_out"


def _sub_ap(full, extra_off, dims):
    """Replace the free dims of a full-tile AP with explicit [stride, count]s."""
    part = [list(full.ap[0])]
    return AP(full.tensor, full.offset + extra_off, part + [list(d) for d in dims])


def build_bass():
    nc = bacc.Bacc(None, target_bir_lowering=False, num_devices=N_CORES)
    inp = nc.declare_dram_parameter(IN_NAME, [P, IN_COLS], mybir.dt.int16, isOutput=False)
    outp = nc.declare_dram_parameter(OUT_NAME, [P, OUT_COLS], mybir.dt.int16, isOutput=True)

    with tile.TileContext(nc) as tc:
        with (
            tc.tile_pool(name="io_in", bufs=5) as pin,
            tc.tile_pool(name="io_out", bufs=4) as pout,
            tc.tile_pool(name="tmp", bufs=3) as ptmp,
        ):
            off = 0
            for ti, W in enumerate(TILE_W):
                tin = pin.tile([P, W * 4], mybir.dt.int16)
                nc.sync.dma_start(tin[:], inp[:, off * 4 : off * 4 + W * 4])
                yx = tin[:, 0 : W * 2]
                hw = tin[:, W * 2 : W * 4]
                yx_rev = _sub_ap(yx, 1, [[2, W], [-1, 2]])  # [x, y] per box
                hw_rev = _sub_ap(hw, 1, [[2, W], [-1, 2]])  # [w, h] per box

                t4 = ptmp.tile([P, W * 4], mybir.dt.int16)
                # t per box = [y, u, v, x]
                nc.vector.tensor_add(
                    _sub_ap(t4[:], 1, [[4, W], [1, 2]]), yx_rev, hw_rev
                )  # t{1,2} = [x+w, y+h] = [u, v]
                nc.scalar.copy(
                    _sub_ap(t4[:], 0, [[4, W], [3, 2]]),
                    yx.rearrange("p (w c) -> p w c", c=2),
                )  # t{0},{3} = [y, x]

                tout = pout.tile([P, W * 8], mybir.dt.int16)
                # out{0,1} = 2*[x, y] = [a, b]   (ACT, reads the input tile)
                nc.scalar.mul(
                    _sub_ap(tout[:], 0, [[8, W], [1, 2]]), yx_rev, 2.0
                )
                # out{4,5} = 2*[u, v] = [c, d]
                nc.vector.tensor_scalar_mul(
                    _sub_ap(tout[:], 4, [[8, W], [1, 2]]),
                    _sub_ap(t4[:], 1, [[4, W], [1, 2]]),
                    2,
                )
                # out{2,3},{6,7} = 2*[u,y],[x,v] = [c,b],[a,d]  (fused rev pairs)
                nc.vector.tensor_scalar_mul(
                    _sub_ap(tout[:], 2, [[8, W], [4, 2], [1, 2]]),
                    _sub_ap(t4[:], 1, [[4, W], [2, 2], [-1, 2]]),
                    2,
                )

                # alternate store rings (ACT HW DGE / Pool SW DGE) so the SP
                # ring stays loads-only and store gaps overlap across rings
                store_eng = nc.scalar if ti % 2 == 0 else nc.gpsimd
                store_eng.dma_start(outp[:, off * 8 : off * 8 + W * 8], tout[:])
                off += W
    nc.compile()
    _strip_entry_barrier(nc)
    return nc


def _strip_entry_barrier(nc):
    """Drop the framework's const-AP all-engine barrier from the entry block.

    Bass.__init__ emits const-AP memsets followed by an all-engine barrier
    (drain + event-sem per engine on the barrier_* gather/release sems).
    This kernel never reads the const APs and all of its own ordering is
    semaphore-based from zero-initialized sems, so the entry rendezvous only
    delays the first load DMA (~2us, gated by the PE warm-up). Only the
    entry block is touched; the tail barriers keep their instructions.
    """
    blk = nc.m.functions[0].blocks[0]
    il = blk.instructions
    keep = []
    dropped = 0
    for ins in il:
        si = getattr(ins, "sync_info", None)
        names = []
        if si is not None:
            names = [w.ant_name or "" for w in si.on_wait] + [
                u.ant_name or "" for u in si.on_update
            ]
        if any(n.startswith("barrier_Pool_Activation_PE_DVE_SP") for n in names):
            dropped += 1
            continue
        keep.append(ins)
    assert dropped == 10, f"expected 10 entry-barrier insts, found {dropped}"
    blk.instructions = keep


_NC_CACHE = []


def _get_nc():
    if not _NC_CACHE:
        _NC_CACHE.append(build_bass())
    return _NC_CACHE[0]


def shard_inputs(boxes: np.ndarray) -> list[dict[str, np.ndarray]]:
    boxes = np.asarray(boxes).astype(np.int16)  # lossless: values in [0, 1000)
    # Per-tile blocked layout: [yx pairs | hw pairs] so device reads are dense.
    v = boxes.reshape(N_CORES, P, BOXES_PER_PART, 2, 2)
    parts = []
    off = 0
    for W in TILE_W:
        blk = v[:, :, off : off + W]  # [8, 128, W, 2, 2]
        parts.append(blk[..., 0, :].reshape(N_CORES, P, W * 2))
        parts.append(blk[..., 1, :].reshape(N_CORES, P, W * 2))
        off += W
    shards = np.ascontiguousarray(np.concatenate(parts, axis=-1))
    return [{IN_NAME: shards[c]} for c in range(N_CORES)]


def unshard_output(per_core: list[np.ndarray]) -> np.ndarray:
    out = np.stack([np.asarray(r) for r in per_core])  # [8, 128, 50000] int16
    return out.reshape(BATCH, NBOX, 4, 2).astype(np.int32)


def kernel(boxes: np.ndarray, **_run_kwargs) -> np.ndarray:
    nc = _get_nc()
    in_maps = shard_inputs(boxes)
    res = run_bass_kernel_spmd(nc, in_maps, list(range(N_CORES)), **_run_kwargs)
    out = unshard_output([res.results[c][OUT_NAME] for c in range(N_CORES)])
    if _run_kwargs:
        kernel.last_results = res
    return out
